# revision 1
# baseline (speedup 1.0000x reference)
"""Trainium2 Bass kernel for nn_DeleteEdgeDecoder.

reference semantics (per batch b):
    feats[e] = [emb[i_e] | emb[j_e] | dist_e]          (513)
    h        = relu(feats @ W1 + b1)                   (E, 512)
    logits   = (h @ W2 + b2)[:, 0]  masked(-inf) + delete_bias

Sharding: pure data parallel, batch dim 128 -> 8 cores x 16.

Device dataflow per batch (this image lacks the custom GPSIMD ucode, so
gathers use the stock [128,1]-offset indirect DGE, 128 rows per call):
  - 16 indirect DMA calls per edge-endpoint side gather fp16 embedding
    rows edge-major into SBUF: g[p, c, :] = emb[idx[c*128+p]]
  - PE transpose-mode flips [128e x 128f] blocks into feature-major
    featsT tiles (fp16 PSUM), ACT copies them to SBUF
  - layer 1: per (h-tile, e-tile), 4 accumulating K=128 matmuls (the
    [emb_i|emb_j] concat is K-accumulation) plus a K=1 rank-1 matmul
    adding w_dist (x) dist into the same PSUM bank
  - ACT: relu(psum + b1) -> h^T fp16 in SBUF
  - layer 2: M=1 matmuls, one PSUM partition-group (0/32/64/96) per
    e-tile, plus a K=1 matmul injecting postbias (valid-mask -inf +
    b2 + delete_bias) into the same accumulation
  - DVE copies the logits PSUM bank to SBUF, DMA out; host extracts
    partition rows 0/32/64/96
"""

import os
from contextlib import ExitStack

import numpy as np
import concourse.bass as bass
import concourse.bacc as bacc
import concourse.mybir as mybir
import concourse.tile as tile
from concourse.bass_utils import run_bass_kernel_spmd

B, N, D, E = 128, 2000, 256, 2000
NCORES = 8
BL = B // NCORES          # batches per core
EP = 2048                 # edges padded to a multiple of 512
H = 512
ET = EP // 512            # 4 e-tiles of 512 edges
HT = H // 128             # 4 h-tiles
KT = (2 * D) // 128       # 4 k-tiles over [emb_i|emb_j]
EC = EP // 128            # 16 gather calls (columns) per side

F16 = mybir.dt.float16
F32 = mybir.dt.float32
I32 = mybir.dt.int32

_CACHE: dict = {}


def _build_nc(bl: int = BL):
    nc = bacc.Bacc(
        "TRN2", target_bir_lowering=False, debug=False, num_devices=NCORES
    )
    emb = nc.dram_tensor("emb", [bl * N, D], F16, kind="ExternalInput")
    idxi = nc.dram_tensor("idxi", [bl, 128, EC], I32, kind="ExternalInput")
    idxj = nc.dram_tensor("idxj", [bl, 128, EC], I32, kind="ExternalInput")
    w1p = nc.dram_tensor("w1p", [128, KT * HT * 128], F16, kind="ExternalInput")
    wd = nc.dram_tensor("wd", [1, H], F16, kind="ExternalInput")
    w2p = nc.dram_tensor("w2p", [128, KT], F16, kind="ExternalInput")
    b1p = nc.dram_tensor("b1p", [128, HT], F32, kind="ExternalInput")
    dist = nc.dram_tensor("dist", [bl, EP], F16, kind="ExternalInput")
    pbias = nc.dram_tensor("pbias", [bl, EP], F16, kind="ExternalInput")
    one = nc.dram_tensor("one", [1, 1], F16, kind="ExternalInput")
    ident = nc.dram_tensor("ident", [128, 128], F16, kind="ExternalInput")
    out = nc.dram_tensor("out", [bl, 128, 512], F32, kind="ExternalOutput")

    with tile.TileContext(nc) as tc, ExitStack() as ctx:
        const = ctx.enter_context(tc.tile_pool(name="const", bufs=1))
        rawp = ctx.enter_context(tc.tile_pool(name="raw", bufs=3))
        ftp = ctx.enter_context(tc.tile_pool(name="ft", bufs=3))
        ipool = ctx.enter_context(tc.tile_pool(name="idx", bufs=3))
        spool = ctx.enter_context(tc.tile_pool(name="small", bufs=3))
        hpool = ctx.enter_context(tc.tile_pool(name="hrelu", bufs=2))
        opool = ctx.enter_context(tc.tile_pool(name="outt", bufs=2))
        psl1 = ctx.enter_context(tc.tile_pool(name="psl1", bufs=4, space="PSUM"))
        pslg = ctx.enter_context(tc.tile_pool(name="pslg", bufs=2, space="PSUM"))
        pstp = ctx.enter_context(tc.tile_pool(name="pstp", bufs=2, space="PSUM"))

        w1_sb = const.tile([128, KT * HT * 128], F16)
        nc.sync.dma_start(w1_sb[:], w1p.ap())
        wd_sb = const.tile([1, H], F16)
        nc.sync.dma_start(wd_sb[:], wd.ap())
        w2_sb = const.tile([128, KT], F16)
        nc.sync.dma_start(w2_sb[:], w2p.ap())
        b1_sb = const.tile([128, HT], F32)
        nc.sync.dma_start(b1_sb[:], b1p.ap())
        one_sb = const.tile([1, 1], F16)
        nc.sync.dma_start(one_sb[:], one.ap())
        id_sb = const.tile([128, 128], F16)
        nc.sync.dma_start(id_sb[:], ident.ap())

        for b in range(bl):
            ii = ipool.tile([128, EC], I32, tag="ii")
            nc.sync.dma_start(ii[:], idxi.ap()[b])
            jj = ipool.tile([128, EC], I32, tag="jj")
            nc.sync.dma_start(jj[:], idxj.ap()[b])
            dist_t = spool.tile([1, EP], F16, tag="dist")
            nc.sync.dma_start(dist_t[:], dist.ap()[b : b + 1, :])
            pb_t = spool.tile([1, EP], F16, tag="pb")
            nc.sync.dma_start(pb_t[:], pbias.ap()[b : b + 1, :])

            # gather edge-major: g[p, c, :] = emb_row(idx[p, c]); e = c*128+p
            gtiles = []
            for side, it in ((0, ii), (1, jj)):
                g = rawp.tile([128, EC, D], F16, tag=f"g{side}")
                for c in range(EC):
                    nc.gpsimd.indirect_dma_start(
                        out=g[:, c, :],
                        out_offset=None,
                        in_=emb.ap(),
                        in_offset=bass.IndirectOffsetOnAxis(
                            ap=it[:, c : c + 1], axis=0
                        ),
                    )
                gtiles.append(g)

            # transpose to feature-major featsT[side][:, dk, e]
            ftiles = []
            for side in range(2):
                ft = ftp.tile([128, D // 128, EP], F16, tag=f"ft{side}")
                for dk in range(D // 128):
                    for oc in range(EC // 8):
                        pt = pstp.tile([128, 8, 128], F16, tag="tp")
                        for c8 in range(8):
                            c = oc * 8 + c8
                            nc.tensor.transpose(
                                pt[:, c8, :],
                                gtiles[side][:, c, dk * 128 : (dk + 1) * 128],
                                id_sb[:],
                            )
                        nc.scalar.copy(
                            ft[:, dk, oc * 1024 : (oc + 1) * 1024],
                            pt[:, :, :],
                        )
                ftiles.append(ft)

            lg = pslg.tile([128, 512], F32, tag="lg")
            nc.vector.memset(lg[:], 0.0)
            for et in range(ET):
                hr = hpool.tile([128, HT, 512], F16, tag="hr")
                es = slice(et * 512, (et + 1) * 512)
                for ht in range(HT):
                    ph = psl1.tile([128, 512], F32, tag="l1")
                    for kt in range(KT):
                        src = ftiles[0] if kt < 2 else ftiles[1]
                        nc.tensor.matmul(
                            ph[:],
                            w1_sb[:, ((kt * HT) + ht) * 128 : ((kt * HT) + ht + 1) * 128],
                            src[:, kt % 2, es],
                            start=(kt == 0),
                            stop=False,
                        )
                    # + w_dist (x) dist  (rank-1, K=1)
                    nc.tensor.matmul(
                        ph[:],
                        wd_sb[0:1, ht * 128 : (ht + 1) * 128],
                        dist_t[0:1, es],
                        start=False,
                        stop=True,
                    )
                    nc.scalar.activation(
                        hr[:, ht, :],
                        ph[:],
                        mybir.ActivationFunctionType.Relu,
                        bias=b1_sb[:, ht : ht + 1],
                    )
                # layer 2 into partition group 32*et of the shared bank
                row = lg[32 * et : 32 * et + 1, :]
                for kt in range(KT):
                    nc.tensor.matmul(
                        row,
                        w2_sb[:, kt : kt + 1],
                        hr[:, kt, :],
                        start=(kt == 0),
                        stop=False,
                        tile_position=(0, 32 * et),
                    )
                # + postbias (valid mask, b2, delete_bias)
                nc.tensor.matmul(
                    row,
                    one_sb[0:1, 0:1],
                    pb_t[0:1, es],
                    start=False,
                    stop=True,
                    tile_position=(0, 32 * et),
                )

            lgs = opool.tile([128, 512], F32, tag="lgs")
            nc.vector.tensor_copy(lgs[:], lg[:])
            nc.sync.dma_start(out.ap()[b], lgs[:])

    nc.compile()
    return nc


def _prep_core_inputs(core, node_embeddings, locs, edge_list, delete_bias,
                      W1, b1, W2, b2, bl: int = BL):
    """Build the per-core input map (layout/dtype marshalling)."""
    b0 = core * bl
    emb16 = node_embeddings[b0 : b0 + bl].astype(np.float16).reshape(bl * N, D)

    el = edge_list[b0 : b0 + bl]  # (bl, E, 2) int
    iclip = np.maximum(el[..., 0], 0).astype(np.int64)
    jclip = np.maximum(el[..., 1], 0).astype(np.int64)
    # global row index into the per-core stacked embedding table
    base = (np.arange(bl, dtype=np.int64) * N)[:, None]
    gi = (iclip + base).astype(np.int32)
    gj = (jclip + base).astype(np.int32)

    def pack(idx):  # (bl, E) -> (bl, 128, EC); tile[p, c] = idx[c*128+p]
        pad = np.zeros((bl, EP), dtype=np.int32)
        pad[:, :E] = idx
        return pad.reshape(bl, EC, 128).transpose(0, 2, 1).copy()

    lc = locs[b0 : b0 + bl]
    bidx = np.arange(bl)[:, None]
    dvec = lc[bidx, iclip] - lc[bidx, jclip]
    dist = np.sqrt((dvec * dvec).sum(-1)).astype(np.float16)  # (bl, E)
    distp = np.zeros((bl, EP), dtype=np.float16)
    distp[:, :E] = dist

    valid = (el[..., 0] >= 0) & (el[..., 1] >= 0)
    pb = np.where(valid, 0.0, -np.inf) + float(np.asarray(b2).reshape(-1)[0]) \
        + float(delete_bias)
    pbp = np.zeros((bl, EP), dtype=np.float16)
    pbp[:, :E] = pb.astype(np.float16)

    w1p = (
        W1[: 2 * D]
        .reshape(KT, 128, HT, 128)
        .transpose(1, 0, 2, 3)
        .reshape(128, KT * HT * 128)
        .astype(np.float16)
    )
    wd = W1[2 * D].reshape(1, H).astype(np.float16)
    w2p = W2[:, 0].reshape(KT, 128).T.astype(np.float16).copy()
    b1p = b1.reshape(HT, 128).T.astype(np.float32).copy()

    return {
        "emb": emb16,
        "idxi": pack(gi),
        "idxj": pack(gj),
        "w1p": np.ascontiguousarray(w1p),
        "wd": np.ascontiguousarray(wd),
        "w2p": w2p,
        "b1p": b1p,
        "dist": distp,
        "pbias": pbp,
        "one": np.ones((1, 1), dtype=np.float16),
        "ident": np.eye(128, dtype=np.float16),
    }


def kernel(node_embeddings, locs, edge_list, delete_bias, W1, b1, W2, b2):
    node_embeddings = np.asarray(node_embeddings, dtype=np.float32)
    locs = np.asarray(locs, dtype=np.float32)
    edge_list = np.asarray(edge_list)
    W1 = np.asarray(W1, dtype=np.float32)
    b1 = np.asarray(b1, dtype=np.float32)
    W2 = np.asarray(W2, dtype=np.float32)
    b2 = np.asarray(b2, dtype=np.float32)

    if "nc" not in _CACHE:
        _CACHE["nc"] = _build_nc()
    nc = _CACHE["nc"]

    in_maps = [
        _prep_core_inputs(c, node_embeddings, locs, edge_list, delete_bias,
                          W1, b1, W2, b2)
        for c in range(NCORES)
    ]
    trace = os.environ.get("BASS_KERNEL_TRACE", "0") == "1"
    res = run_bass_kernel_spmd(nc, in_maps, list(range(NCORES)), trace=trace)
    _CACHE["last_result"] = res

    outs = []
    for c in range(NCORES):
        o = np.asarray(res.results[c]["out"], dtype=np.float32)
        o = o.reshape(BL, 128, 512)[:, ::32, :].reshape(BL, EP)
        outs.append(o[:, :E])
    return np.concatenate(outs, axis=0)

